# revision 3
# baseline (speedup 1.0000x reference)
"""Sinusoidal positional-encoding kernel for Trainium2 (8 NeuronCores).

Output = sinusoidal PE table (seq_len=8192, emb_dim=1024) broadcast to
x.shape = (4, 8192, 1024).  The values of x are unused (only shape/dtype).

Sharding: sequence dim split across 8 cores (1024 positions each); the
batch broadcast is a zero-copy view on the host.

Per-core algorithm (positions are split into 8 blocks of 128 = partitions):
  * A rank-5 fp32 matmul computes, for every output element j=(g, 2k+half),
      t[p, j] = r_c(p) * v[k] + phi[s(p)][j]
    where r_c(p) = (p mod 32) - 15.5, s(p) = p div 32, v = w_k/(2*pi), and
    phi = host-precomputed *wrapped* phase (includes the +0.25 turn that
    turns sin into cos for odd columns).  For k >= KFIX the result is
    already in [-0.566, 0.566] turns, inside the HW Sin spline domain.
  * For k < KFIX columns the angle can be up to ~3 turns, so the DVE does a
    magic-number range reduction: u = t + 1.5*2^23 ; negf = (u - M) - t.
  * ScalarE Sin activation evaluates sin(2*pi*t) (scale=+-2*pi) directly --
    sin/cos interleaving is already baked into phi, so no strided APs.
  * DMA out 512 KB per position-block.
"""

import sys

for _p in ("/opt/trn_rl_repo", "/root/.axon_site/_ro/trn_rl_repo"):
    if _p not in sys.path:
        sys.path.insert(0, _p)

import numpy as np

B, S, D = 4, 8192, 1024
N_CORES = 8
S_CORE = S // N_CORES          # 1024 positions per core
G = S_CORE // 128              # 8 position blocks (partition tiles)
KF = D // 2                    # 512 frequencies
SEG = 32                       # positions per phi segment (B=32)
NSEG = 128 // SEG              # 4 segments -> K = 5
KRANK = 1 + NSEG
BASE = 10000.0
MAGIC = float(1.5 * 2**23)
# k >= KFIX: |t| <= 15.5*v_k + 0.5 <= 0.5658 turns (3.555 rad Sin domain)
KFIX = 202
CFIX = 2 * KFIX                # first CFIX columns of each g-block need fixup

_cache = {}


def _build_bass():
    import concourse.bacc as bacc
    import concourse.mybir as mybir
    from concourse.tile import TileContext

    nc = bacc.Bacc("TRN2", target_bir_lowering=False, debug=False)
    f32 = mybir.dt.float32
    lhsT = nc.dram_tensor("lhsT", [KRANK, 128], f32, kind="ExternalInput")
    rhs = nc.dram_tensor("rhs", [KRANK, G * D], f32, kind="ExternalInput")
    out = nc.dram_tensor("out", [S_CORE, D], f32, kind="ExternalOutput")

    two_pi = float(2 * np.pi)
    Sin = mybir.ActivationFunctionType.Sin
    M = mybir.AluOpType

    with TileContext(nc) as tc:
        with (
            tc.tile_pool(name="const", bufs=1) as cpool,
            tc.tile_pool(name="work", bufs=3) as pool,
            tc.tile_pool(name="psum", bufs=3, space="PSUM") as ppool,
        ):
            tl = cpool.tile([KRANK, 128], f32)
            tr = cpool.tile([KRANK, G * D], f32)
            nc.sync.dma_start(tl[:], lhsT[:])
            nc.sync.dma_start(tr[:], rhs[:])
            for g in range(G):
                tp = ppool.tile([128, D], f32)
                nc.tensor.matmul(
                    tp[:, 0:512], tl[:], tr[:, g * D : g * D + 512],
                    start=True, stop=True,
                )
                nc.tensor.matmul(
                    tp[:, 512:D], tl[:], tr[:, g * D + 512 : (g + 1) * D],
                    start=True, stop=True,
                )
                pe = pool.tile([128, D], f32, tag="pe")
                # k < KFIX columns: range-reduce on DVE then Sin(-2pi * negf)
                tu = pool.tile([128, CFIX], f32, tag="u")
                tn = pool.tile([128, CFIX], f32, tag="negf")
                nc.vector.tensor_scalar(
                    tu[:], tp[:, 0:CFIX], MAGIC, None, op0=M.add
                )
                nc.vector.scalar_tensor_tensor(
                    tn[:], tu[:], MAGIC, tp[:, 0:CFIX],
                    op0=M.subtract, op1=M.subtract,
                )
                nc.scalar.activation(pe[:, 0:CFIX], tn[:], Sin, scale=-two_pi)
                # k >= KFIX columns: already reduced, straight from PSUM
                nc.scalar.activation(
                    pe[:, CFIX:D], tp[:, CFIX:D], Sin, scale=two_pi
                )
                nc.sync.dma_start(
                    out[g * 128 : (g + 1) * 128, :], pe[:]
                )
    nc.compile()
    return nc


def _tables():
    """lhsT (shared) and per-core rhs with wrapped phases."""
    k = np.arange(KF, dtype=np.float64)
    v64 = np.float_power(BASE, -2.0 * k / D) / (2 * np.pi)
    p = np.arange(128)
    lh = np.zeros((KRANK, 128), np.float32)
    lh[0, :] = (p % SEG) - (SEG - 1) / 2.0
    for s in range(NSEG):
        lh[1 + s, s * SEG : (s + 1) * SEG] = 1.0

    ks = np.arange(D) // 2          # column -> frequency index
    halves = np.arange(D) % 2       # 0 = sin, 1 = cos
    vcol = v64[ks]
    rhs_all = []
    for c in range(N_CORES):
        rh = np.zeros((KRANK, G * D), np.float64)
        for g in range(G):
            sl = slice(g * D, (g + 1) * D)
            rh[0, sl] = vcol
            for s in range(NSEG):
                center = c * S_CORE + g * 128 + s * SEG + (SEG - 1) / 2.0
                ph = center * vcol + 0.25 * halves
                rh[1 + s, sl] = ph - np.round(ph)
        rhs_all.append(rh.astype(np.float32))
    return lh, rhs_all


def kernel(x: np.ndarray, base) -> np.ndarray:
    assert tuple(x.shape) == (B, S, D) and int(base) == int(BASE)
    from concourse.bass_utils import run_bass_kernel_spmd

    if "nc" not in _cache:
        _cache["nc"] = _build_bass()
        _cache["tables"] = _tables()
    nc = _cache["nc"]
    lh, rhs_all = _cache["tables"]
    in_maps = [{"lhsT": lh, "rhs": rhs_all[c]} for c in range(N_CORES)]
    res = run_bass_kernel_spmd(nc, in_maps, core_ids=list(range(N_CORES)))
    full = np.concatenate([r["out"] for r in res.results], axis=0)
    out = np.broadcast_to(full[None], (B, S, D))
    return out.astype(np.float32, copy=False)


if __name__ == "__main__":
    x = np.zeros((B, S, D), np.float32)
    r = kernel(x=x, base=10000)
    print(r.shape, r.dtype, r[0, :3, :4])


# revision 9
# speedup vs baseline: 1.6400x; 1.6400x over previous
"""Sinusoidal positional-encoding kernel for Trainium2 (8 NeuronCores).

Output = sinusoidal PE table (seq_len=8192, emb_dim=1024) broadcast to
x.shape = (4, 8192, 1024).  The values of x are unused (only shape/dtype).

Sharding: sequence dim split across 8 cores (1024 positions each); the
batch broadcast is a zero-copy view on the host.

Per-core algorithm (positions are split into 8 blocks of 128 = partitions):
  * A rank-15 bf16 matmul computes, for every output element j=(g, 2k+half),
      t[p, j] = r_c(p) * v[k] + phi[s(p)][j]
    where r_c(p) = (p mod 32) - 15.5, s(p) = p div 32, v = w_k/(2*pi), and
    phi = host-precomputed *wrapped* phase (includes the +0.25 turn that
    turns sin into cos for odd columns).  v and phi are each split into
    three bf16 components (lhsT rows are bf16-exact half-integers and 0/1
    indicators; bf16*bf16 products are exact in the fp32 PSUM accumulate),
    so the result keeps fp32-level precision while the PE streams at bf16
    rate (fp32 matmul is ~5x slower).  For k >= KFIX the result is already
    in [-0.566, 0.566] turns, inside the HW Sin spline domain.
  * For k < KFIX columns the angle can be up to ~3 turns, so the DVE does a
    magic-number range reduction: u = t + 1.5*2^23 ; negf = (u - M) - t.
  * ScalarE Sin activation evaluates sin(2*pi*t) (scale=+-2*pi) directly --
    sin/cos interleaving is already baked into phi, so no strided APs.
  * DMA out 512 KB per position-block.
"""

import sys

for _p in ("/opt/trn_rl_repo", "/root/.axon_site/_ro/trn_rl_repo"):
    if _p not in sys.path:
        sys.path.insert(0, _p)

import numpy as np

B, S, D = 4, 8192, 1024
N_CORES = 8
S_CORE = S // N_CORES          # 1024 positions per core
G = S_CORE // 128              # 8 position blocks (partition tiles)
KF = D // 2                    # 512 frequencies
SEG = 32                       # positions per phi segment (B=32)
NSEG = 128 // SEG              # 4 segments
NSPLIT = 3                     # bf16 splits per value
KRANK = 16                     # 15 used rows (3 splits x 5 terms) + 1 pad
BASE = 10000.0
MAGIC = float(1.5 * 2**23)
# k >= KFIX: |t| <= 15.5*v_k + 0.5 <= 0.5658 turns (3.555 rad Sin domain)
KFIX = 202
CFIX = 2 * KFIX                # first CFIX columns of each g-block need fixup

_cache = {}


def _build_bass():
    import concourse.bacc as bacc
    import concourse.mybir as mybir
    from concourse.tile import TileContext

    nc = bacc.Bacc("TRN2", target_bir_lowering=False, debug=False)
    f32 = mybir.dt.float32
    bf16 = mybir.dt.bfloat16
    lhsT = nc.dram_tensor("lhsT", [KRANK, 128], bf16, kind="ExternalInput")
    rhs = nc.dram_tensor("rhs", [KRANK, G * D], bf16, kind="ExternalInput")
    out = nc.dram_tensor("out", [S_CORE, D], f32, kind="ExternalOutput")

    two_pi = float(2 * np.pi)
    Sin = mybir.ActivationFunctionType.Sin
    M = mybir.AluOpType

    with TileContext(nc) as tc:
        with (
            tc.tile_pool(name="const", bufs=1) as cpool,
            tc.tile_pool(name="work", bufs=3) as pool,
            tc.tile_pool(name="psum", bufs=3, space="PSUM") as ppool,
        ):
            tl = cpool.tile([KRANK, 128], bf16)
            tr = cpool.tile([KRANK, G * D], bf16)
            nc.sync.dma_start(tl[:], lhsT[:])
            nc.sync.dma_start(tr[:], rhs[:])
            for g in range(G):
                tp = ppool.tile([128, D], f32)
                nc.tensor.matmul(
                    tp[:, 0:512], tl[:], tr[:, g * D : g * D + 512],
                    start=True, stop=True,
                )
                nc.tensor.matmul(
                    tp[:, 512:D], tl[:], tr[:, g * D + 512 : (g + 1) * D],
                    start=True, stop=True,
                )
                pe = pool.tile([128, D], f32, tag="pe")
                # k < KFIX columns: range-reduce on DVE then Sin(-2pi * negf)
                tu = pool.tile([128, CFIX], f32, tag="u")
                tn = pool.tile([128, CFIX], f32, tag="negf")
                nc.vector.tensor_scalar(
                    tu[:], tp[:, 0:CFIX], MAGIC, None, op0=M.add
                )
                nc.vector.scalar_tensor_tensor(
                    tn[:], tu[:], MAGIC, tp[:, 0:CFIX],
                    op0=M.subtract, op1=M.subtract,
                )
                nc.scalar.activation(pe[:, 0:CFIX], tn[:], Sin, scale=-two_pi)
                # k >= KFIX columns: already reduced, straight from PSUM
                nc.scalar.activation(
                    pe[:, CFIX:D], tp[:, CFIX:D], Sin, scale=two_pi
                )
                nc.sync.dma_start(
                    out[g * 128 : (g + 1) * 128, :], pe[:]
                )
    nc.compile()
    return nc


def _split3(x64):
    """Split fp64 values into 3 bf16 components summing to ~fp32 accuracy."""
    import ml_dtypes

    bf = ml_dtypes.bfloat16
    p1 = x64.astype(bf)
    r1 = x64 - p1.astype(np.float64)
    p2 = r1.astype(bf)
    r2 = r1 - p2.astype(np.float64)
    p3 = r2.astype(bf)
    return p1, p2, p3


def _tables():
    """lhsT (shared) and per-core rhs with wrapped phases, bf16 3-split.

    Row layout (KRANK = 15): rows 3*t+{0,1,2} are the three bf16 splits of
    term t, where term 0 is r_c*v and terms 1+s are the segment phases.
    """
    import ml_dtypes

    bf = ml_dtypes.bfloat16
    k = np.arange(KF, dtype=np.float64)
    v64 = np.float_power(BASE, -2.0 * k / D) / (2 * np.pi)
    p = np.arange(128)
    lh = np.zeros((KRANK, 128), bf)
    rc = ((p % SEG) - (SEG - 1) / 2.0).astype(np.float64)  # bf16-exact values
    for j in range(NSPLIT):
        lh[j, :] = rc.astype(bf)
    for s in range(NSEG):
        for j in range(NSPLIT):
            lh[NSPLIT * (1 + s) + j, s * SEG : (s + 1) * SEG] = bf(1.0)

    ks = np.arange(D) // 2          # column -> frequency index
    halves = np.arange(D) % 2       # 0 = sin, 1 = cos
    vcol = v64[ks]
    v1, v2, v3 = _split3(vcol)
    rhs_all = []
    for c in range(N_CORES):
        rh = np.zeros((KRANK, G * D), bf)
        for g in range(G):
            sl = slice(g * D, (g + 1) * D)
            rh[0, sl], rh[1, sl], rh[2, sl] = v1, v2, v3
            for s in range(NSEG):
                center = c * S_CORE + g * 128 + s * SEG + (SEG - 1) / 2.0
                ph = center * vcol + 0.25 * halves
                ph = ph - np.round(ph)
                b = NSPLIT * (1 + s)
                rh[b, sl], rh[b + 1, sl], rh[b + 2, sl] = _split3(ph)
        rhs_all.append(rh)
    return lh, rhs_all


def kernel(x: np.ndarray, base) -> np.ndarray:
    assert tuple(x.shape) == (B, S, D) and int(base) == int(BASE)
    from concourse.bass_utils import run_bass_kernel_spmd

    if "nc" not in _cache:
        _cache["nc"] = _build_bass()
        _cache["tables"] = _tables()
    nc = _cache["nc"]
    lh, rhs_all = _cache["tables"]
    in_maps = [{"lhsT": lh, "rhs": rhs_all[c]} for c in range(N_CORES)]
    res = run_bass_kernel_spmd(nc, in_maps, core_ids=list(range(N_CORES)))
    full = np.concatenate([r["out"] for r in res.results], axis=0)
    out = np.broadcast_to(full[None], (B, S, D))
    return out.astype(np.float32, copy=False)


if __name__ == "__main__":
    x = np.zeros((B, S, D), np.float32)
    r = kernel(x=x, base=10000)
    print(r.shape, r.dtype, r[0, :3, :4])


# revision 10
# speedup vs baseline: 1.6429x; 1.0017x over previous
"""Sinusoidal positional-encoding kernel for Trainium2 (8 NeuronCores).

Output = sinusoidal PE table (seq_len=8192, emb_dim=1024) broadcast to
x.shape = (4, 8192, 1024).  The values of x are unused (only shape/dtype).

Sharding: sequence dim split across 8 cores (1024 positions each); the
batch broadcast is a zero-copy view on the host.

Per-core algorithm (positions are split into 8 blocks of 128 = partitions):
  * A rank-15 bf16 matmul computes, for every output element j=(g, 2k+half),
      t[p, j] = r_c(p) * v[k] + phi[s(p)][j]
    where r_c(p) = (p mod 32) - 15.5, s(p) = p div 32, v = w_k/(2*pi), and
    phi = host-precomputed *wrapped* phase (includes the +0.25 turn that
    turns sin into cos for odd columns).  v and phi are each split into
    three bf16 components (lhsT rows are bf16-exact half-integers and 0/1
    indicators; bf16*bf16 products are exact in the fp32 PSUM accumulate),
    so the result keeps fp32-level precision while the PE streams at bf16
    rate (fp32 matmul is ~5x slower).  For k >= KFIX the result is already
    in [-0.566, 0.566] turns, inside the HW Sin spline domain.
  * For k < KFIX columns the angle can be up to ~3 turns, so the DVE does a
    magic-number range reduction: u = t + 1.5*2^23 ; negf = (u - M) - t.
  * ScalarE Sin activation evaluates sin(2*pi*t) (scale=+-2*pi) directly --
    sin/cos interleaving is already baked into phi, so no strided APs.
  * DMA out 512 KB per position-block.
"""

import sys

for _p in ("/opt/trn_rl_repo", "/root/.axon_site/_ro/trn_rl_repo"):
    if _p not in sys.path:
        sys.path.insert(0, _p)

import numpy as np

B, S, D = 4, 8192, 1024
N_CORES = 8
S_CORE = S // N_CORES          # 1024 positions per core
G = S_CORE // 128              # 8 position blocks (partition tiles)
KF = D // 2                    # 512 frequencies
SEG = 32                       # positions per phi segment (B=32)
NSEG = 128 // SEG              # 4 segments
NSPLIT = 3                     # bf16 splits per value
KRANK = 16                     # 15 used rows (3 splits x 5 terms) + 1 pad
BASE = 10000.0
MAGIC = float(1.5 * 2**23)
# k >= KFIX: |t| <= 15.5*v_k + 0.5 <= 0.5658 turns (3.555 rad Sin domain)
KFIX = 202
CFIX = 2 * KFIX                # first CFIX columns of each g-block need fixup

_cache = {}


def _build_bass():
    import concourse.bacc as bacc
    import concourse.mybir as mybir
    from concourse.tile import TileContext

    nc = bacc.Bacc("TRN2", target_bir_lowering=False, debug=False)
    f32 = mybir.dt.float32
    bf16 = mybir.dt.bfloat16
    lhsT = nc.dram_tensor("lhsT", [KRANK, 128], bf16, kind="ExternalInput")
    rhs = nc.dram_tensor("rhs", [KRANK, G * D], bf16, kind="ExternalInput")
    out = nc.dram_tensor("out", [S_CORE, D], f32, kind="ExternalOutput")

    two_pi = float(2 * np.pi)
    Sin = mybir.ActivationFunctionType.Sin
    M = mybir.AluOpType

    with TileContext(nc) as tc:
        with (
            tc.tile_pool(name="const", bufs=1) as cpool,
            tc.tile_pool(name="work", bufs=4) as pool,
            tc.tile_pool(name="psum", bufs=4, space="PSUM") as ppool,
        ):
            tl = cpool.tile([KRANK, 128], bf16)
            nc.sync.dma_start(tl[:], lhsT[:])
            # per-g rhs tiles so the first matmul starts after 1/8 of the load
            trs = []
            for g in range(G):
                tr = cpool.tile([KRANK, D], bf16, tag=f"rhs{g}")
                nc.sync.dma_start(tr[:], rhs[:, g * D : (g + 1) * D])
                trs.append(tr)
            for g in range(G):
                tr = trs[g]
                tp = ppool.tile([128, D], f32)
                nc.tensor.matmul(
                    tp[:, 0:512], tl[:], tr[:, 0:512], start=True, stop=True
                )
                nc.tensor.matmul(
                    tp[:, 512:D], tl[:], tr[:, 512:D], start=True, stop=True
                )
                # k < KFIX columns hold unreduced turns t (up to ~3): compute
                # n = round(t) on DVE, subtract in place in PSUM.
                tn = pool.tile([128, CFIX], f32, tag="n")
                nc.vector.tensor_scalar(
                    tn[:], tp[:, 0:CFIX], MAGIC, MAGIC,
                    op0=M.add, op1=M.subtract,
                )
                nc.vector.tensor_tensor(
                    tp[:, 0:CFIX], tp[:, 0:CFIX], tn[:], op=M.subtract
                )
                pe = pool.tile([128, D], f32, tag="pe")
                nc.scalar.activation(pe[:], tp[:], Sin, scale=two_pi)
                nc.sync.dma_start(
                    out[g * 128 : (g + 1) * 128, :], pe[:]
                )
    nc.compile()
    return nc


def _split3(x64):
    """Split fp64 values into 3 bf16 components summing to ~fp32 accuracy."""
    import ml_dtypes

    bf = ml_dtypes.bfloat16
    p1 = x64.astype(bf)
    r1 = x64 - p1.astype(np.float64)
    p2 = r1.astype(bf)
    r2 = r1 - p2.astype(np.float64)
    p3 = r2.astype(bf)
    return p1, p2, p3


def _tables():
    """lhsT (shared) and per-core rhs with wrapped phases, bf16 3-split.

    Row layout (KRANK = 15): rows 3*t+{0,1,2} are the three bf16 splits of
    term t, where term 0 is r_c*v and terms 1+s are the segment phases.
    """
    import ml_dtypes

    bf = ml_dtypes.bfloat16
    k = np.arange(KF, dtype=np.float64)
    v64 = np.float_power(BASE, -2.0 * k / D) / (2 * np.pi)
    p = np.arange(128)
    lh = np.zeros((KRANK, 128), bf)
    rc = ((p % SEG) - (SEG - 1) / 2.0).astype(np.float64)  # bf16-exact values
    for j in range(NSPLIT):
        lh[j, :] = rc.astype(bf)
    for s in range(NSEG):
        for j in range(NSPLIT):
            lh[NSPLIT * (1 + s) + j, s * SEG : (s + 1) * SEG] = bf(1.0)

    ks = np.arange(D) // 2          # column -> frequency index
    halves = np.arange(D) % 2       # 0 = sin, 1 = cos
    vcol = v64[ks]
    v1, v2, v3 = _split3(vcol)
    rhs_all = []
    for c in range(N_CORES):
        rh = np.zeros((KRANK, G * D), bf)
        for g in range(G):
            sl = slice(g * D, (g + 1) * D)
            rh[0, sl], rh[1, sl], rh[2, sl] = v1, v2, v3
            for s in range(NSEG):
                center = c * S_CORE + g * 128 + s * SEG + (SEG - 1) / 2.0
                ph = center * vcol + 0.25 * halves
                ph = ph - np.round(ph)
                b = NSPLIT * (1 + s)
                rh[b, sl], rh[b + 1, sl], rh[b + 2, sl] = _split3(ph)
        rhs_all.append(rh)
    return lh, rhs_all


def kernel(x: np.ndarray, base) -> np.ndarray:
    assert tuple(x.shape) == (B, S, D) and int(base) == int(BASE)
    from concourse.bass_utils import run_bass_kernel_spmd

    if "nc" not in _cache:
        _cache["nc"] = _build_bass()
        _cache["tables"] = _tables()
    nc = _cache["nc"]
    lh, rhs_all = _cache["tables"]
    in_maps = [{"lhsT": lh, "rhs": rhs_all[c]} for c in range(N_CORES)]
    res = run_bass_kernel_spmd(nc, in_maps, core_ids=list(range(N_CORES)))
    full = np.concatenate([r["out"] for r in res.results], axis=0)
    out = np.broadcast_to(full[None], (B, S, D))
    return out.astype(np.float32, copy=False)


if __name__ == "__main__":
    x = np.zeros((B, S, D), np.float32)
    r = kernel(x=x, base=10000)
    print(r.shape, r.dtype, r[0, :3, :4])
